# revision 1
# baseline (speedup 1.0000x reference)
"""GroupedQueryAttention Trainium2 kernel (8 NeuronCores).

Sharding: (batch b in 0..1) x (kv-head group g in 0..3) -> core 4*b+g.
Each core computes, for its batch, the 4 query heads (4g..4g+3) that share
kv head g, plus the partial output projection through the matching 512-row
slice of Wo.  The host sums the 4 partials per batch.

On-device dataflow is fully "transposed": activations live as [feature,
token] so every matmul contraction sits on the partition axis, and the
softmax probabilities come out directly in the layout the P@V matmul
needs (no on-chip transposes of the attention matrix).  Softmax
denominators come from an all-ones stationary matmul over the same
probability tiles, which also yields them pre-broadcast across partitions.
Causality is exploited by only computing score tiles on/below the block
diagonal; the block-diagonal tiles are masked with a 0/1 step+triangle
pattern after the exp.
"""

import numpy as np
import ml_dtypes

DIM, H, KV, S, B = 2048, 16, 4, 2048, 2
HD = DIM // H          # 128
GQ = H // KV           # 4 query heads per kv head
P = 128                # partitions
NK = DIM // P          # 16 contraction tiles
NCH = S // 512         # 4 sequence chunks of 512
EPS = 1e-6
BF = ml_dtypes.bfloat16

_CACHED = {}


def _build_program():
    import concourse.bass as bass
    import concourse.tile as tile
    from concourse import bacc
    from concourse import mybir
    from concourse.masks import make_identity

    f32 = mybir.dt.float32
    bf16 = mybir.dt.bfloat16
    AF = mybir.ActivationFunctionType

    nc = bacc.Bacc()
    xT = nc.declare_dram_parameter("xT", [DIM, S], bf16, isOutput=False)
    wq = nc.declare_dram_parameter("wq", [DIM, GQ * HD], bf16, isOutput=False)
    wk = nc.declare_dram_parameter("wk", [DIM, HD], bf16, isOutput=False)
    wv = nc.declare_dram_parameter("wv", [DIM, HD], bf16, isOutput=False)
    wo = nc.declare_dram_parameter("wo", [GQ * HD, DIM], bf16, isOutput=False)
    cosq = nc.declare_dram_parameter("cosq", [HD, S], bf16, isOutput=False)
    sinq = nc.declare_dram_parameter("sinq", [HD, S], bf16, isOutput=False)
    cosk = nc.declare_dram_parameter("cosk", [HD, S], bf16, isOutput=False)
    sink = nc.declare_dram_parameter("sink", [HD, S], bf16, isOutput=False)
    m4 = nc.declare_dram_parameter("m4", [4, P, 512], bf16, isOutput=False)
    rsw = nc.declare_dram_parameter("rsw", [P, P], bf16, isOutput=False)
    po = nc.declare_dram_parameter("po", [S, DIM], f32, isOutput=True)

    inv_sqrt_hd = 1.0 / float(np.sqrt(HD))

    with tile.TileContext(nc) as tc:
      with tc.tile_pool(name="const", bufs=1) as const, \
           tc.tile_pool(name="hatp", bufs=1) as hatp, \
           tc.tile_pool(name="w5", bufs=2) as w5, \
           tc.tile_pool(name="m4p", bufs=1) as m4p, \
           tc.tile_pool(name="csp", bufs=1) as csp:
        ones_sb = const.tile([P, P], bf16)
        nc.vector.memset(ones_sb, 1.0)
        ident = const.tile([P, P], bf16)
        make_identity(nc, ident)
        epsb = const.tile([P, 1], f32)
        nc.vector.memset(epsb, EPS)
        rsw_sb = const.tile([P, P], bf16)
        nc.scalar.dma_start(out=rsw_sb, in_=rsw[:, :])

        # prefetched during P1 (emitted after chunk 0 so they queue behind it)
        wo_sb = w5.tile([P, GQ, DIM], bf16, bufs=1)
        m4_sb = m4p.tile([P, 4, 512], bf16)
        cs_sb = {}
        for nm in ("cosq", "sinq", "cosk", "sink"):
            cs_sb[nm] = csp.tile([P, S], bf16, tag=f"cs_{nm}", name=f"cs_{nm}")

        v_nat = hatp.tile([P, NK, HD], bf16, tag="vnat")
        onorm = [hatp.tile([P, S], bf16, tag=f"onorm{h}", name=f"onorm{h}")
                 for h in range(GQ)]
        qhat = [hatp.tile([P, S], bf16, tag=f"qhat{h}", name=f"qhat{h}")
                for h in range(GQ)]
        khat = hatp.tile([P, S], bf16, tag="khat")

        with tc.tile_pool(name="qkvp", bufs=1) as qkvp:
            q32 = [qkvp.tile([P, S], bf16, tag=f"qp_{h}", name=f"qp_{h}")
                   for h in range(GQ)]
            k32 = qkvp.tile([P, S], bf16, tag="kp")
            vT_bf = qkvp.tile([P, S], bf16, tag="vT")

            # ---- P1: projections (transposed outputs) + v transpose ----
            with tc.tile_pool(name="xt", bufs=1) as xtp, \
                 tc.tile_pool(name="p1ps", bufs=3, space="PSUM") as p1ps:
                wk_sb = xtp.tile([P, NK, HD], bf16)
                nc.sync.dma_start(out=wk_sb, in_=wk.ap().rearrange("(j p) n -> p j n", p=P))
                xt_sb = xtp.tile([P, NK, S], bf16)
                xt_src = xT.ap().rearrange("(j p) t -> p j t", p=P)
                nc.sync.dma_start(out=xt_sb[:, :, 0:512], in_=xt_src[:, :, 0:512])
                wq_sb = xtp.tile([P, NK, GQ * HD], bf16)
                nc.sync.dma_start(out=wq_sb, in_=wq.ap().rearrange("(j p) n -> p j n", p=P))
                wv_sb = xtp.tile([P, NK, HD], bf16)
                nc.sync.dma_start(out=wv_sb, in_=wv.ap().rearrange("(j p) n -> p j n", p=P))
                for c in range(1, NCH):
                    csl = slice(c * 512, (c + 1) * 512)
                    nc.sync.dma_start(out=xt_sb[:, :, csl], in_=xt_src[:, :, csl])
                # k first: khat is needed by every score tile in P3
                for c in range(NCH):
                    sl = slice(c * 512, (c + 1) * 512)
                    for slot in (4, 0, 1, 2, 3, 5):
                        ps = p1ps.tile([P, 512], f32, tag="proj")
                        for j in range(NK):
                            if slot < 4:
                                lhs = wq_sb[:, j, slot * HD:(slot + 1) * HD]
                            elif slot == 4:
                                lhs = wk_sb[:, j, :]
                            else:
                                lhs = wv_sb[:, j, :]
                            nc.tensor.matmul(ps, lhs, xt_sb[:, j, sl],
                                             start=(j == 0), stop=(j == NK - 1))
                        if slot < 4:
                            nc.scalar.copy(q32[slot][:, sl], ps)
                        elif slot == 4:
                            nc.scalar.copy(k32[:, sl], ps)
                        else:
                            nc.scalar.copy(vT_bf[:, sl], ps)
                nc.scalar.dma_start(out=wo_sb, in_=wo.ap().rearrange("(h p) n -> p h n", p=P))
                nc.scalar.dma_start(out=m4_sb, in_=m4.ap().rearrange("a p n -> p a n"))
                for nm, t in (("cosq", cosq), ("sinq", sinq), ("cosk", cosk), ("sink", sink)):
                    nc.scalar.dma_start(out=cs_sb[nm], in_=t[:, :])
                # v natural layout [sk_local, j, d] via PE transpose of vT
                for j in range(NK):
                    tp = p1ps.tile([P, HD], bf16, tag="vtr")
                    nc.tensor.transpose(tp, vT_bf[:, j * HD:(j + 1) * HD], ident)
                    nc.scalar.copy(v_nat[:, j, :], tp)

            # ---- P2: rmsnorm (pre-gain) + rope, full-row ops, k first ----
            with tc.tile_pool(name="w2", bufs=2) as w2, \
                 tc.tile_pool(name="p2ps", bufs=2, space="PSUM") as p2ps:
                for t in (4, 0, 1, 2, 3):
                    src = q32[t] if t < 4 else k32
                    dst = qhat[t] if t < 4 else khat
                    cosT = cs_sb["cosq" if t < 4 else "cosk"]
                    sinT = cs_sb["sinq" if t < 4 else "sink"]
                    # sum of squares over feature (partition) axis via
                    # all-ones matmul; arrives replicated on all partitions
                    sqb = w2.tile([P, S], bf16, tag="sqb")
                    nc.scalar.activation(sqb, src, AF.Square)
                    ssq = p2ps.tile([P, S], f32, tag="ssq", bufs=1)
                    rot = p2ps.tile([P, S // 2], f32, tag="rot", bufs=2)
                    rot2 = p2ps.tile([P, S // 2], f32, tag="rot", bufs=2)
                    for c in range(NCH):
                        sl = slice(c * 512, (c + 1) * 512)
                        nc.tensor.matmul(ssq[:, sl], ones_sb, sqb[:, sl],
                                         start=True, stop=True)
                        rt = rot if c < 2 else rot2
                        rsl = slice((c % 2) * 512, (c % 2 + 1) * 512)
                        nc.tensor.matmul(rt[:, rsl], rsw_sb, src[:, sl],
                                         start=True, stop=True)
                    lnb = w2.tile([P, S], f32, tag="lnb")
                    nc.scalar.activation(lnb, ssq, AF.Ln, bias=epsb, scale=1.0 / HD)
                    rsb = w2.tile([P, S], bf16, tag="rsb")
                    nc.scalar.activation(rsb, lnb, AF.Exp, scale=-0.5)
                    # rope: y = src*cos + rot(src)*sin (sign/gain in tables)
                    t1 = w2.tile([P, S], bf16, tag="t1")
                    nc.vector.tensor_mul(t1, src, cosT)
                    t2 = w2.tile([P, S], bf16, tag="t2")
                    nc.vector.tensor_mul(t2[:, 0:1024], rot, sinT[:, 0:1024])
                    nc.vector.tensor_mul(t2[:, 1024:2048], rot2, sinT[:, 1024:2048])
                    t3 = w2.tile([P, S], bf16, tag="t3")
                    nc.vector.tensor_add(t3, t1, t2)
                    nc.vector.tensor_mul(dst, t3, rsb)

        # ---- P3: attention, all heads per chunk ----
        with tc.tile_pool(name="wep", bufs=2) as wep:
          with tc.tile_pool(name="ptp", bufs=36) as ptp, \
               tc.tile_pool(name="p3s", bufs=2, space="PSUM") as p3s, \
               tc.tile_pool(name="p3o", bufs=4, space="PSUM") as p3o:
              for c in range(NCH):
                  sl = slice(c * 512, (c + 1) * 512)
                  nj = 4 * c + 4
                  # scores + exp, two 512-wide sk-tiles per PSUM tile so the
                  # exp runs 1024 wide
                  ptsc = {}
                  for h in range(GQ):
                      for pr in range(nj // 2):
                          sc = p3s.tile([P, 1024], f32, tag="sc",
                                        name=f"sc_{c}_{h}_{pr}")
                          for u in range(2):
                              j = 2 * pr + u
                              nc.tensor.matmul(sc[:, u * 512:(u + 1) * 512],
                                               khat[:, j * P:(j + 1) * P],
                                               qhat[h][:, sl],
                                               start=True, stop=True)
                          pt = ptp.tile([P, 1024], bf16, tag="pt",
                                        name=f"pt_{c}_{h}_{pr}")
                          nc.scalar.activation(pt, sc, AF.Exp, scale=inv_sqrt_hd)
                          for u in range(2):
                              j = 2 * pr + u
                              if j >= 4 * c:
                                  usl = slice(u * 512, (u + 1) * 512)
                                  nc.vector.tensor_mul(pt[:, usl], pt[:, usl],
                                                   m4_sb[:, j - 4 * c, :])
                          ptsc[(h, pr)] = pt
                  # P@V, j-outer so the stationary v tile is reused across heads
                  ots = [p3o.tile([P, 512], f32, tag="ot", name=f"ot_{c}_{h}")
                         for h in range(GQ)]
                  for j in range(nj):
                      usl = slice((j % 2) * 512, (j % 2 + 1) * 512)
                      for h in range(GQ):
                          nc.tensor.matmul(ots[h], v_nat[:, j, :],
                                           ptsc[(h, j // 2)][:, usl],
                                           start=(j == 0), stop=(j == nj - 1))
                  # denominators (replicated across partitions by the all-ones
                  # stationary; den tiles reuse the sc slots), then normalize
                  for h in range(GQ):
                      den = p3s.tile([P, 512], f32, tag="sc", name=f"den_{c}_{h}")
                      for j in range(nj):
                          usl = slice((j % 2) * 512, (j % 2 + 1) * 512)
                          nc.tensor.matmul(den, ones_sb,
                                           ptsc[(h, j // 2)][:, usl],
                                           start=(j == 0), stop=(j == nj - 1))
                      rec = wep.tile([P, 512], f32, tag="rec")
                      nc.vector.reciprocal(rec, den)
                      nc.vector.tensor_mul(onorm[h][:, sl], ots[h], rec)

          # ---- P5: partial output projection: po = onorm^T @ Wo_g ----
          with tc.tile_pool(name="p5ps", bufs=5, space="PSUM") as p5ps:
              for i in range(S // P):
                  isl = slice(i * P, (i + 1) * P)
                  po_ps = [p5ps.tile([P, 512], f32, tag="po", name=f"po_{i}_{n2}")
                           for n2 in range(NCH)]
                  for h in range(GQ):
                      for n in range(NCH):
                          nc.tensor.matmul(po_ps[n], onorm[h][:, isl],
                                           wo_sb[:, h, n * 512:(n + 1) * 512],
                                           start=(h == 0), stop=(h == GQ - 1))
                  row = wep.tile([P, DIM], f32, tag="row")
                  for n in range(NCH):
                      if n % 2 == 0:
                          nc.scalar.copy(row[:, n * 512:(n + 1) * 512], po_ps[n])
                      else:
                          nc.vector.tensor_copy(row[:, n * 512:(n + 1) * 512], po_ps[n])
                  nc.sync.dma_start(out=po[isl, :], in_=row)
    nc.compile()
    return nc


def _causal_ok(mask):
    m = np.asarray(mask).reshape(S, S)
    tri = np.tril(np.ones((S, S), dtype=bool))
    return bool(np.all(m[tri] == 0.0) and np.all(m[~tri] <= -1e8))


def _reference_fallback(x, Wq, Wk, Wv, Wo, qg, kg, cos, sin, mask):
    x64 = np.asarray(x, dtype=np.float32)
    q = (x64 @ Wq).reshape(B, S, H, HD).transpose(0, 2, 1, 3)
    k = (x64 @ Wk).reshape(B, S, KV, HD).transpose(0, 2, 1, 3)
    v = (x64 @ Wv).reshape(B, S, KV, HD).transpose(0, 2, 1, 3)

    def rms(t, g):
        r = np.sqrt(np.mean(t * t, axis=-1, keepdims=True) + EPS)
        return g * (t / r)

    q, k = rms(q, qg), rms(k, kg)

    def rot(t):
        return np.concatenate([-t[..., HD // 2:], t[..., :HD // 2]], axis=-1)

    c = cos[None, None, :, :]
    s = sin[None, None, :, :]
    q = q * c + rot(q) * s
    k = k * c + rot(k) * s
    k = np.repeat(k, GQ, axis=1)
    v = np.repeat(v, GQ, axis=1)
    sc = np.einsum('bhqd,bhkd->bhqk', q, k) / np.sqrt(HD) + np.asarray(mask).reshape(1, 1, S, S)
    sc = sc - sc.max(axis=-1, keepdims=True)
    e = np.exp(sc)
    a = e / e.sum(axis=-1, keepdims=True)
    o = np.einsum('bhqk,bhkd->bhqd', a, v)
    o = o.transpose(0, 2, 1, 3).reshape(B, S, H * HD)
    return (o @ Wo).astype(np.float32)


def kernel(x, Wq, Wk, Wv, Wo, qg, kg, cos, sin, mask, **_unused):
    x = np.asarray(x, dtype=np.float32)
    Wq, Wk, Wv, Wo = (np.asarray(a, dtype=np.float32) for a in (Wq, Wk, Wv, Wo))
    qg, kg = np.asarray(qg, np.float32), np.asarray(kg, np.float32)
    cos, sin = np.asarray(cos, np.float32), np.asarray(sin, np.float32)
    if not _causal_ok(mask):
        return _reference_fallback(x, Wq, Wk, Wv, Wo, qg, kg, cos, sin, mask)

    from concourse.bass_utils import run_bass_kernel_spmd

    if "nc" not in _CACHED:
        _CACHED["nc"] = _build_program()
    nc = _CACHED["nc"]

    cosT = np.ascontiguousarray(cos.T)  # [HD, S]
    sinT = np.ascontiguousarray(sin.T)

    # rope via halves: out[:64] = x[:64]*cos[:64] + x[64:]*sin_tbl[:64]
    #                  out[64:] = x[64:]*cos[64:] + x[:64]*sin_tbl[64:]
    # reference: rot(x)[:64] = -x[64:], rot(x)[64:] = x[:64]; gains fold in.
    def tables(g):
        ct = cosT * g[:, None]
        st = np.empty_like(sinT)
        st[:64] = -sinT[:64] * g[64:, None]
        st[64:] = sinT[64:] * g[:64, None]
        return ct.astype(BF), st.astype(BF)

    cq, sq = tables(qg)
    ck, sk = tables(kg)

    rsw = np.zeros((P, P), dtype=np.float32)
    for i in range(P):
        rsw[i, (i + 64) % P] = 1.0
    rsw = rsw.astype(BF)

    cols = np.arange(512)[None, :]
    rows = np.arange(P)[:, None]
    m4 = np.stack([(cols - P * a >= rows) for a in range(4)]).astype(BF)

    xT = [np.ascontiguousarray(x[b].T).astype(BF) for b in range(B)]

    in_maps = []
    for core in range(8):
        b, g = divmod(core, KV)
        in_maps.append({
            "xT": xT[b],
            "wq": np.ascontiguousarray(Wq[:, g * GQ * HD:(g + 1) * GQ * HD]).astype(BF),
            "wk": np.ascontiguousarray(Wk[:, g * HD:(g + 1) * HD]).astype(BF),
            "wv": np.ascontiguousarray(Wv[:, g * HD:(g + 1) * HD]).astype(BF),
            "wo": np.ascontiguousarray(Wo[g * GQ * HD:(g + 1) * GQ * HD, :]).astype(BF),
            "cosq": cq, "sinq": sq, "cosk": ck, "sink": sk,
            "m4": m4, "rsw": rsw,
        })

    res = run_bass_kernel_spmd(nc, in_maps, list(range(8)))
    out = np.zeros((B, S, DIM), dtype=np.float32)
    for core in range(8):
        out[core // KV] += res.results[core]["po"]
    return out



# revision 2
# speedup vs baseline: 1.1562x; 1.1562x over previous
"""GroupedQueryAttention Trainium2 kernel (8 NeuronCores).

Sharding: (batch b in 0..1) x (kv-head group g in 0..3) -> core 4*b+g.
Each core computes, for its batch, the 4 query heads (4g..4g+3) that share
kv head g, plus the partial output projection through the matching 512-row
slice of Wo.  The host sums the 4 bf16 partials per batch in f32.

On-device dataflow is fully "transposed": activations live as [feature,
token] so every matmul contraction sits on the partition axis, and the
softmax probabilities come out directly in the layout the P@V matmul
needs.  Performance structure vs the phase-serial baseline:
  - host pre-arranges every DRAM operand so each DMA is contiguous per
    partition; x arrives in per-chunk pieces on the sync queue while the
    weight tensors stream on the scalar queue
  - dummy warm-up matmuls run during the initial DMA wait so the PE HAM
    clock gate is at 8/8 when real work starts
  - causal diagonal 512-blocks are computed on restricted query ranges
    (512/384/256/128 wide) instead of full width + mask
  - softmax denominators are accumulated on the Vector engine (bf16 adds
    over the probability tiles) with a single ones-matmul partition
    reduction per (chunk, head); 1/den uses reciprocal_approx_fast
  - rmsnorm uses reciprocal_approx_fast + a Sqrt activation, keeping the
    Scalar engine on two activation-table sets total (sqrt phase, exp
    phase) instead of thrashing Square/Ln/Exp loads
  - the output projection is interleaved per chunk with attention, with
    PSUM pools sized so attention + projection coexist in the 8 banks;
    bf16 rows stream out as soon as each 128-token tile is projected
"""

import numpy as np
import ml_dtypes

DIM, H, KV, S, B = 2048, 16, 4, 2048, 2
HD = DIM // H          # 128
GQ = H // KV           # 4 query heads per kv head
P = 128                # partitions
NK = DIM // P          # 16 contraction tiles
NCH = S // 512         # 4 sequence chunks of 512
EPS = 1e-6
BF = ml_dtypes.bfloat16

_CACHED = {}


def _build_program():
    import concourse.bass as bass
    import concourse.tile as tile
    from concourse import bacc
    from concourse import mybir
    from concourse.masks import make_identity

    f32 = mybir.dt.float32
    bf16 = mybir.dt.bfloat16
    AF = mybir.ActivationFunctionType

    nc = bacc.Bacc()
    xt4 = nc.declare_dram_parameter("xt4", [P, NCH, NK, 512], bf16, isOutput=False)
    wq = nc.declare_dram_parameter("wq", [P, NK, GQ * HD], bf16, isOutput=False)
    wk = nc.declare_dram_parameter("wk", [P, NK, HD], bf16, isOutput=False)
    wv = nc.declare_dram_parameter("wv", [P, NK, HD], bf16, isOutput=False)
    wo = nc.declare_dram_parameter("wo", [P, GQ, DIM], bf16, isOutput=False)
    cosq = nc.declare_dram_parameter("cosq", [HD, S], bf16, isOutput=False)
    sinq = nc.declare_dram_parameter("sinq", [HD, S], bf16, isOutput=False)
    cosk = nc.declare_dram_parameter("cosk", [HD, S], bf16, isOutput=False)
    sink = nc.declare_dram_parameter("sink", [HD, S], bf16, isOutput=False)
    mtri = nc.declare_dram_parameter("mtri", [P, 1280], bf16, isOutput=False)
    rsw = nc.declare_dram_parameter("rsw", [P, P], bf16, isOutput=False)
    po = nc.declare_dram_parameter("po", [S, DIM], bf16, isOutput=True)

    inv_sqrt_hd = 1.0 / float(np.sqrt(HD))

    with tile.TileContext(nc) as tc:
      with tc.tile_pool(name="const", bufs=1) as const, \
           tc.tile_pool(name="w5", bufs=1) as w5, \
           tc.tile_pool(name="hatp", bufs=1) as hatp:
        ones_sb = const.tile([P, P], bf16)
        nc.vector.memset(ones_sb, 1.0)
        osb = const.tile([P, P], bf16)           # 1/HD for the rmsnorm mean
        nc.vector.memset(osb, 1.0 / HD)
        ident = const.tile([P, P], bf16)
        make_identity(nc, ident)
        wmov = const.tile([P, 512], bf16)
        nc.vector.memset(wmov, 0.0)
        mtri_sb = const.tile([P, 1280], bf16)
        nc.scalar.dma_start(out=mtri_sb, in_=mtri[:, :])
        sq_dummy = const.tile([P, 1], f32)
        nc.vector.memset(sq_dummy, 1.0)
        sq_dummy_o = const.tile([P, 1], bf16)
        # preload the sqrt activation-table set during the DMA wait
        nc.scalar.activation(sq_dummy_o, sq_dummy, AF.Sqrt)

        wo_sb = w5.tile([P, GQ, DIM], bf16)
        nc.scalar.dma_start(out=wo_sb, in_=wo.ap().rearrange("p h n -> p (h n)"))

        v_nat = hatp.tile([P, NK, HD], bf16, tag="vnat")
        khat = hatp.tile([P, S], bf16, tag="khat")
        qhat = [hatp.tile([P, S], bf16, tag=f"qhat{h}", name=f"qhat{h}")
                for h in range(GQ)]
        onorm = [hatp.tile([P, S], bf16, tag=f"onorm{h}", name=f"onorm{h}")
                 for h in range(GQ)]

        # ---- warm-up: keep the PE busy while the first DMAs land ----
        with tc.tile_pool(name="wps", bufs=1, space="PSUM") as wps:
            wt = wps.tile([P, 512], f32, tag="warm")
            for _ in range(14):
                nc.tensor.matmul(wt, ones_sb, wmov, start=True, stop=True)

        # ---- phase A: projections + rmsnorm + rope, chunk-pipelined ----
        with tc.tile_pool(name="xtp", bufs=1) as xtp, \
             tc.tile_pool(name="xchk", bufs=2) as xchk, \
             tc.tile_pool(name="q32p", bufs=8) as q32p, \
             tc.tile_pool(name="vTp", bufs=2) as vTp, \
             tc.tile_pool(name="scr", bufs=2) as scr, \
             tc.tile_pool(name="psA", bufs=3, space="PSUM") as psA, \
             tc.tile_pool(name="psA2", bufs=2, space="PSUM") as psA2:
            wk_sb = xtp.tile([P, NK, HD], bf16, tag="wk")
            nc.sync.dma_start(out=wk_sb, in_=wk.ap().rearrange("p j n -> p (j n)"))
            wq_sb = xtp.tile([P, NK, GQ * HD], bf16, tag="wq")
            nc.scalar.dma_start(out=wq_sb, in_=wq.ap().rearrange("p j n -> p (j n)"))
            wv_sb = xtp.tile([P, NK, HD], bf16, tag="wv")
            nc.scalar.dma_start(out=wv_sb, in_=wv.ap().rearrange("p j n -> p (j n)"))
            rsw_sb = xtp.tile([P, P], bf16, tag="rsw")
            nc.scalar.dma_start(out=rsw_sb, in_=rsw[:, :])
            cs_sb = {}
            for nm, t in (("cosq", cosq), ("sinq", sinq), ("cosk", cosk), ("sink", sink)):
                cs_sb[nm] = xtp.tile([P, S], bf16, tag=f"cs_{nm}", name=f"cs_{nm}")
                nc.scalar.dma_start(out=cs_sb[nm], in_=t[:, :])

            for c in range(NCH):
                sl = slice(c * 512, (c + 1) * 512)
                xt_c = xchk.tile([P, NK, 512], bf16, tag="xt", name=f"xt{c}")
                nc.sync.dma_start(out=xt_c, in_=xt4.ap()[:, c].rearrange("p j n -> p (j n)"))
                # projections: k first (khat feeds every chunk's scores),
                # then v (feeds PV), then the 4 q heads
                srcs = {}
                for slot in (4, 5, 0, 1, 2, 3):
                    ps = psA.tile([P, 512], f32, tag="proj")
                    for j in range(NK):
                        if slot < 4:
                            lhs = wq_sb[:, j, slot * HD:(slot + 1) * HD]
                        elif slot == 4:
                            lhs = wk_sb[:, j, :]
                        else:
                            lhs = wv_sb[:, j, :]
                        nc.tensor.matmul(ps, lhs, xt_c[:, j, :],
                                         start=(j == 0), stop=(j == NK - 1))
                    if slot == 5:
                        vT_c = vTp.tile([P, 512], bf16, tag="vT")
                        nc.scalar.copy(vT_c, ps)
                        for u in range(4):
                            tp = psA2.tile([P, HD], bf16, tag="vtr")
                            nc.tensor.transpose(tp, vT_c[:, u * HD:(u + 1) * HD], ident)
                            nc.scalar.copy(v_nat[:, 4 * c + u, :], tp)
                    else:
                        t32 = q32p.tile([P, 512], bf16, tag="q32",
                                        name=f"q32_{c}_{slot}")
                        nc.scalar.copy(t32, ps)
                        srcs[slot] = t32
                # rmsnorm + rope for this chunk (k first)
                for t in (4, 0, 1, 2, 3):
                    src = srcs[t]
                    dst = qhat[t] if t < 4 else khat
                    cosT = cs_sb["cosq" if t < 4 else "cosk"]
                    sinT = cs_sb["sinq" if t < 4 else "sink"]
                    sqb = scr.tile([P, 512], bf16, tag="sqb")
                    nc.vector.tensor_mul(sqb, src, src)
                    ssq = psA.tile([P, 512], f32, tag="proj")
                    nc.tensor.matmul(ssq, osb, sqb, start=True, stop=True)
                    msinv = scr.tile([P, 512], f32, tag="msinv")
                    nc.vector.reciprocal_approx_fast(out=msinv, in_=ssq)
                    rsb = scr.tile([P, 512], bf16, tag="rsb")
                    nc.scalar.activation(rsb, msinv, AF.Sqrt)
                    rot = psA2.tile([P, 512], f32, tag="rot")
                    nc.tensor.matmul(rot, rsw_sb, src, start=True, stop=True)
                    t1 = scr.tile([P, 512], bf16, tag="t1")
                    nc.vector.tensor_mul(t1, src, cosT[:, sl])
                    t2 = scr.tile([P, 512], bf16, tag="t2")
                    nc.vector.tensor_mul(t2, rot, sinT[:, sl])
                    t3 = scr.tile([P, 512], bf16, tag="t3")
                    nc.vector.tensor_add(t3, t1, t2)
                    nc.vector.tensor_mul(dst[:, sl], t3, rsb)

        # ---- phase B: attention + output projection, per chunk ----
        with tc.tile_pool(name="ptp", bufs=16) as ptp, \
             tc.tile_pool(name="accp", bufs=2) as accp, \
             tc.tile_pool(name="recp", bufs=2) as recp, \
             tc.tile_pool(name="rowp", bufs=2) as rowp, \
             tc.tile_pool(name="psc", bufs=2, space="PSUM") as psc, \
             tc.tile_pool(name="pss", bufs=4, space="PSUM") as pss:
            for c in range(NCH):
                sl = slice(c * 512, (c + 1) * 512)
                for h in range(GQ):
                    # scores -> exp -> (mask) -> probability tiles
                    # off-diagonal key-tile pairs: full 512-query width
                    pv_list = []
                    for pr in range(2 * c):
                        sc = psc.tile([P, 1024], f32, tag="sc",
                                      name=f"sc_{c}_{h}_{pr}")
                        for u in range(2):
                            j = 2 * pr + u
                            nc.tensor.matmul(sc[:, u * 512:(u + 1) * 512],
                                             khat[:, j * P:(j + 1) * P],
                                             qhat[h][:, sl],
                                             start=True, stop=True)
                        pt = ptp.tile([P, 1024], bf16, tag="pt",
                                      name=f"pt_{c}_{h}_{pr}")
                        nc.scalar.activation(pt, sc, AF.Exp, scale=inv_sqrt_hd)
                        pv_list.append((2 * pr, pt, 0, 0, 512))
                        pv_list.append((2 * pr + 1, pt, 512, 0, 512))
                    # diagonal 512-block: restricted query ranges
                    # tile u covers queries [128u, 512) of the chunk
                    scA = psc.tile([P, 1024], f32, tag="sc", name=f"scA_{c}_{h}")
                    nc.tensor.matmul(scA[:, 0:512],
                                     khat[:, (4 * c) * P:(4 * c + 1) * P],
                                     qhat[h][:, c * 512:(c + 1) * 512],
                                     start=True, stop=True)
                    nc.tensor.matmul(scA[:, 512:896],
                                     khat[:, (4 * c + 1) * P:(4 * c + 2) * P],
                                     qhat[h][:, c * 512 + 128:(c + 1) * 512],
                                     start=True, stop=True)
                    ptA = ptp.tile([P, 1024], bf16, tag="pt", name=f"ptA_{c}_{h}")
                    nc.scalar.activation(ptA[:, 0:896], scA[:, 0:896],
                                         AF.Exp, scale=inv_sqrt_hd)
                    nc.vector.tensor_mul(ptA[:, 0:896], ptA[:, 0:896],
                                         mtri_sb[:, 0:896])
                    scB = psc.tile([P, 1024], f32, tag="sc", name=f"scB_{c}_{h}")
                    nc.tensor.matmul(scB[:, 0:256],
                                     khat[:, (4 * c + 2) * P:(4 * c + 3) * P],
                                     qhat[h][:, c * 512 + 256:(c + 1) * 512],
                                     start=True, stop=True)
                    nc.tensor.matmul(scB[:, 256:384],
                                     khat[:, (4 * c + 3) * P:(4 * c + 4) * P],
                                     qhat[h][:, c * 512 + 384:(c + 1) * 512],
                                     start=True, stop=True)
                    ptB = ptp.tile([P, 1024], bf16, tag="pt", name=f"ptB_{c}_{h}")
                    nc.scalar.activation(ptB[:, 0:384], scB[:, 0:384],
                                         AF.Exp, scale=inv_sqrt_hd)
                    nc.vector.tensor_mul(ptB[:, 0:384], ptB[:, 0:384],
                                         mtri_sb[:, 896:1280])
                    pv_list.append((4 * c + 0, ptA, 0, 0, 512))
                    pv_list.append((4 * c + 1, ptA, 512, 128, 384))
                    pv_list.append((4 * c + 2, ptB, 0, 256, 256))
                    pv_list.append((4 * c + 3, ptB, 256, 384, 128))

                    # denominator: accumulate probability tiles on DVE,
                    # then one ones-matmul partition reduction
                    acc = accp.tile([P, 512], bf16, tag="acc", name=f"acc_{c}_{h}")
                    first = True
                    for (_, pt, co, qo, w) in pv_list:
                        if first:
                            nc.vector.tensor_copy(acc, pt[:, co:co + w])
                            first = False
                        else:
                            nc.vector.tensor_add(acc[:, qo:qo + w],
                                                 acc[:, qo:qo + w],
                                                 pt[:, co:co + w])
                    den = pss.tile([P, 512], f32, tag="s", name=f"den_{c}_{h}")
                    nc.tensor.matmul(den, ones_sb, acc, start=True, stop=True)
                    rec = recp.tile([P, 512], f32, tag="rec")
                    nc.vector.reciprocal_approx_fast(out=rec, in_=den)

                    # P@V for this head, j-ordered accumulation
                    ots = pss.tile([P, 512], f32, tag="s", name=f"ot_{c}_{h}")
                    n_pv = len(pv_list)
                    for idx, (j, pt, co, qo, w) in enumerate(pv_list):
                        nc.tensor.matmul(ots[:, qo:qo + w], v_nat[:, j, :],
                                         pt[:, co:co + w],
                                         start=(idx == 0), stop=(idx == n_pv - 1))
                    nc.vector.tensor_mul(onorm[h][:, sl], ots, rec)

                # output projection for this chunk's 4 token tiles
                for i in range(4 * c, 4 * c + 4):
                    isl = slice(i * P, (i + 1) * P)
                    row = rowp.tile([P, DIM], bf16, tag="row", name=f"row_{i}")
                    for n in range(NCH):
                        po_ps = pss.tile([P, 512], f32, tag="s",
                                         name=f"po_{i}_{n}")
                        for h in range(GQ):
                            nc.tensor.matmul(po_ps, onorm[h][:, isl],
                                             wo_sb[:, h, n * 512:(n + 1) * 512],
                                             start=(h == 0), stop=(h == GQ - 1))
                        nc.vector.tensor_copy(row[:, n * 512:(n + 1) * 512], po_ps)
                    nc.sync.dma_start(out=po[isl, :], in_=row)
    nc.compile()
    return nc


def _causal_ok(mask):
    m = np.asarray(mask).reshape(S, S)
    tri = np.tril(np.ones((S, S), dtype=bool))
    return bool(np.all(m[tri] == 0.0) and np.all(m[~tri] <= -1e8))


def _reference_fallback(x, Wq, Wk, Wv, Wo, qg, kg, cos, sin, mask):
    x64 = np.asarray(x, dtype=np.float32)
    q = (x64 @ Wq).reshape(B, S, H, HD).transpose(0, 2, 1, 3)
    k = (x64 @ Wk).reshape(B, S, KV, HD).transpose(0, 2, 1, 3)
    v = (x64 @ Wv).reshape(B, S, KV, HD).transpose(0, 2, 1, 3)

    def rms(t, g):
        r = np.sqrt(np.mean(t * t, axis=-1, keepdims=True) + EPS)
        return g * (t / r)

    q, k = rms(q, qg), rms(k, kg)

    def rot(t):
        return np.concatenate([-t[..., HD // 2:], t[..., :HD // 2]], axis=-1)

    c = cos[None, None, :, :]
    s = sin[None, None, :, :]
    q = q * c + rot(q) * s
    k = k * c + rot(k) * s
    k = np.repeat(k, GQ, axis=1)
    v = np.repeat(v, GQ, axis=1)
    sc = np.einsum('bhqd,bhkd->bhqk', q, k) / np.sqrt(HD) + np.asarray(mask).reshape(1, 1, S, S)
    sc = sc - sc.max(axis=-1, keepdims=True)
    e = np.exp(sc)
    a = e / e.sum(axis=-1, keepdims=True)
    o = np.einsum('bhqk,bhkd->bhqd', a, v)
    o = o.transpose(0, 2, 1, 3).reshape(B, S, H * HD)
    return (o @ Wo).astype(np.float32)


def kernel(x, Wq, Wk, Wv, Wo, qg, kg, cos, sin, mask, **_unused):
    x = np.asarray(x, dtype=np.float32)
    Wq, Wk, Wv, Wo = (np.asarray(a, dtype=np.float32) for a in (Wq, Wk, Wv, Wo))
    qg, kg = np.asarray(qg, np.float32), np.asarray(kg, np.float32)
    cos, sin = np.asarray(cos, np.float32), np.asarray(sin, np.float32)
    if not _causal_ok(mask):
        return _reference_fallback(x, Wq, Wk, Wv, Wo, qg, kg, cos, sin, mask)

    from concourse.bass_utils import run_bass_kernel_spmd

    if "nc" not in _CACHED:
        _CACHED["nc"] = _build_program()
    nc = _CACHED["nc"]

    cosT = np.ascontiguousarray(cos.T)  # [HD, S]
    sinT = np.ascontiguousarray(sin.T)

    # rope via halves: out[:64] = x[:64]*cos[:64] + x[64:]*sin_tbl[:64]
    #                  out[64:] = x[64:]*cos[64:] + x[:64]*sin_tbl[64:]
    # reference: rot(x)[:64] = -x[64:], rot(x)[64:] = x[:64]; gains fold in.
    def tables(g):
        ct = cosT * g[:, None]
        st = np.empty_like(sinT)
        st[:64] = -sinT[:64] * g[64:, None]
        st[64:] = sinT[64:] * g[:64, None]
        return ct.astype(BF), st.astype(BF)

    cq, sq = tables(qg)
    ck, sk = tables(kg)

    rsw = np.zeros((P, P), dtype=np.float32)
    for i in range(P):
        rsw[i, (i + 64) % P] = 1.0
    rsw = rsw.astype(BF)

    # restricted-diagonal masks: within each 128-column sub-range that
    # starts a diagonal tile, query-col >= key-row; elsewhere 1.
    rows = np.arange(P)[:, None]
    tri = (np.arange(P)[None, :] >= rows)          # [128,128] step
    onesP = np.ones((P, P), dtype=bool)
    mA = np.concatenate([tri, onesP, onesP, onesP, tri, onesP, onesP], axis=1)  # 896
    mB = np.concatenate([tri, onesP, tri], axis=1)                              # 384
    mtri = np.concatenate([mA, mB], axis=1).astype(BF)                          # [128,1280]

    def part_layout(w, cols):
        # [DIM, cols] -> [P, NK, cols] with feature d = j*128 + p
        return np.ascontiguousarray(w.reshape(NK, P, cols).transpose(1, 0, 2)).astype(BF)

    xt4 = []
    for b in range(B):
        xT = x[b].T  # [DIM, S]
        xt4.append(np.ascontiguousarray(
            xT.reshape(NK, P, NCH, 512).transpose(1, 2, 0, 3)).astype(BF))

    in_maps = []
    for core in range(8):
        b, g = divmod(core, KV)
        wo_g = Wo[g * GQ * HD:(g + 1) * GQ * HD, :]
        in_maps.append({
            "xt4": xt4[b],
            "wq": part_layout(Wq[:, g * GQ * HD:(g + 1) * GQ * HD], GQ * HD),
            "wk": part_layout(Wk[:, g * HD:(g + 1) * HD], HD),
            "wv": part_layout(Wv[:, g * HD:(g + 1) * HD], HD),
            "wo": np.ascontiguousarray(
                wo_g.reshape(GQ, P, DIM).transpose(1, 0, 2)).astype(BF),
            "cosq": cq, "sinq": sq, "cosk": ck, "sink": sk,
            "mtri": mtri, "rsw": rsw,
        })

    res = run_bass_kernel_spmd(nc, in_maps, list(range(8)))
    out = np.zeros((B, S, DIM), dtype=np.float32)
    for core in range(8):
        out[core // KV] += res.results[core]["po"].astype(np.float32)
    return out


# revision 12
# speedup vs baseline: 1.1564x; 1.0002x over previous
"""GroupedQueryAttention Trainium2 kernel (8 NeuronCores).

Sharding: (batch b in 0..1) x (kv-head group g in 0..3) -> core 4*b+g.
Each core computes, for its batch, the 4 query heads (4g..4g+3) that share
kv head g, plus the partial output projection through the matching 512-row
slice of Wo.  The host sums the 4 bf16 partials per batch in f32.

On-device dataflow is fully "transposed": activations live as [feature,
token] so every matmul contraction sits on the partition axis, and the
softmax probabilities come out directly in the layout the P@V matmul
needs.  Performance structure vs the phase-serial baseline:
  - host pre-arranges every DRAM operand so each DMA is contiguous per
    partition; x arrives in per-chunk pieces on the sync queue while the
    weight tensors stream on the scalar queue
  - dummy warm-up matmuls run during the initial DMA wait so the PE HAM
    clock gate is at 8/8 when real work starts
  - causal diagonal 512-blocks are computed on restricted query ranges
    (512/384/256/128 wide) instead of full width + mask
  - softmax denominators are accumulated on the Vector engine (bf16 adds
    over the probability tiles) with a single ones-matmul partition
    reduction per (chunk, head); 1/den uses reciprocal_approx_fast
  - rmsnorm uses reciprocal_approx_fast + a Sqrt activation, keeping the
    Scalar engine on two activation-table sets total (sqrt phase, exp
    phase) instead of thrashing Square/Ln/Exp loads
  - the output projection is interleaved per chunk with attention, with
    PSUM pools sized so attention + projection coexist in the 8 banks;
    bf16 rows stream out as soon as each 128-token tile is projected
"""

import numpy as np
import ml_dtypes

DIM, H, KV, S, B = 2048, 16, 4, 2048, 2
HD = DIM // H          # 128
GQ = H // KV           # 4 query heads per kv head
P = 128                # partitions
NK = DIM // P          # 16 contraction tiles
NCH = S // 512         # 4 sequence chunks of 512
EPS = 1e-6
BF = ml_dtypes.bfloat16

_CACHED = {}


def _build_program():
    import concourse.bass as bass
    import concourse.tile as tile
    from concourse import bacc
    from concourse import mybir
    from concourse.masks import make_identity

    f32 = mybir.dt.float32
    bf16 = mybir.dt.bfloat16
    AF = mybir.ActivationFunctionType

    nc = bacc.Bacc()
    xt4 = nc.declare_dram_parameter("xt4", [P, NCH, NK, 512], bf16, isOutput=False)
    wq = nc.declare_dram_parameter("wq", [P, NK, GQ * HD], bf16, isOutput=False)
    wk = nc.declare_dram_parameter("wk", [P, NK, HD], bf16, isOutput=False)
    wv = nc.declare_dram_parameter("wv", [P, NK, HD], bf16, isOutput=False)
    wo = nc.declare_dram_parameter("wo", [P, GQ, DIM], bf16, isOutput=False)
    cosq = nc.declare_dram_parameter("cosq", [HD, S], bf16, isOutput=False)
    sinq = nc.declare_dram_parameter("sinq", [HD, S], bf16, isOutput=False)
    cosk = nc.declare_dram_parameter("cosk", [HD, S], bf16, isOutput=False)
    sink = nc.declare_dram_parameter("sink", [HD, S], bf16, isOutput=False)
    mtri = nc.declare_dram_parameter("mtri", [P, 1280], bf16, isOutput=False)
    rsw = nc.declare_dram_parameter("rsw", [P, P], bf16, isOutput=False)
    po = nc.declare_dram_parameter("po", [S, DIM], bf16, isOutput=True)

    inv_sqrt_hd = 1.0 / float(np.sqrt(HD))

    with tile.TileContext(nc) as tc:
      with tc.tile_pool(name="const", bufs=1) as const, \
           tc.tile_pool(name="w5", bufs=1) as w5, \
           tc.tile_pool(name="hatp", bufs=1) as hatp:
        ones_sb = const.tile([P, P], bf16)
        nc.vector.memset(ones_sb, 1.0)
        osb = const.tile([P, P], bf16)           # 1/HD for the rmsnorm mean
        nc.vector.memset(osb, 1.0 / HD)
        ident = const.tile([P, P], bf16)
        make_identity(nc, ident)
        wmov = const.tile([P, 512], bf16)
        nc.vector.memset(wmov, 0.0)
        mtri_sb = const.tile([P, 1280], bf16)
        nc.gpsimd.dma_start(out=mtri_sb, in_=mtri[:, :])
        sq_dummy = const.tile([P, 1], f32)
        nc.vector.memset(sq_dummy, 1.0)
        sq_dummy_o = const.tile([P, 1], bf16)
        # preload the sqrt activation-table set during the DMA wait
        nc.scalar.activation(sq_dummy_o, sq_dummy, AF.Sqrt)

        wo_sb = w5.tile([P, GQ, DIM], bf16)
        nc.gpsimd.dma_start(out=wo_sb, in_=wo.ap().rearrange("p h n -> p (h n)"))

        v_nat = hatp.tile([P, NK, HD], bf16, tag="vnat")
        khat = hatp.tile([P, S], bf16, tag="khat")
        qhat = [hatp.tile([P, S], bf16, tag=f"qhat{h}", name=f"qhat{h}")
                for h in range(GQ)]
        onorm = [hatp.tile([P, S], bf16, tag=f"onorm{h}", name=f"onorm{h}")
                 for h in range(GQ)]

        # ---- warm-up: keep the PE busy while the first DMAs land ----
        with tc.tile_pool(name="wps", bufs=1, space="PSUM") as wps:
            wt = wps.tile([P, 512], f32, tag="warm")
            for _ in range(14):
                nc.tensor.matmul(wt, ones_sb, wmov, start=True, stop=True)

        # ---- phase A: projections + rmsnorm + rope, chunk-pipelined ----
        with tc.tile_pool(name="xtp", bufs=1) as xtp, \
             tc.tile_pool(name="xchk", bufs=2) as xchk, \
             tc.tile_pool(name="q32p", bufs=12) as q32p, \
             tc.tile_pool(name="vTp", bufs=2) as vTp, \
             tc.tile_pool(name="scr", bufs=2) as scr, \
             tc.tile_pool(name="psA", bufs=3, space="PSUM") as psA, \
             tc.tile_pool(name="psA2", bufs=2, space="PSUM") as psA2:
            wk_sb = xtp.tile([P, NK, HD], bf16, tag="wk")
            nc.sync.dma_start(out=wk_sb, in_=wk.ap().rearrange("p j n -> p (j n)"))
            wq_sb = xtp.tile([P, NK, GQ * HD], bf16, tag="wq")
            nc.scalar.dma_start(out=wq_sb, in_=wq.ap().rearrange("p j n -> p (j n)"))
            wv_sb = xtp.tile([P, NK, HD], bf16, tag="wv")
            nc.scalar.dma_start(out=wv_sb, in_=wv.ap().rearrange("p j n -> p (j n)"))
            rsw_sb = xtp.tile([P, P], bf16, tag="rsw")
            nc.gpsimd.dma_start(out=rsw_sb, in_=rsw[:, :])
            cs_sb = {}
            for nm, t in (("cosq", cosq), ("sinq", sinq), ("cosk", cosk), ("sink", sink)):
                cs_sb[nm] = xtp.tile([P, S], bf16, tag=f"cs_{nm}", name=f"cs_{nm}")
                nc.gpsimd.dma_start(out=cs_sb[nm], in_=t[:, :])

            for c in range(NCH):
                sl = slice(c * 512, (c + 1) * 512)
                xt_c = xchk.tile([P, NK, 512], bf16, tag="xt", name=f"xt{c}")
                # split across both HWDGE queues so neither caps the rate
                xsrc = xt4.ap()[:, c]
                for hq in range(4):
                    jsl = slice(hq * 4, (hq + 1) * 4)
                    eng = nc.sync if hq % 2 == 0 else nc.scalar
                    eng.dma_start(out=xt_c[:, jsl, :], in_=xsrc[:, jsl])
                # projections: k first (khat feeds every chunk's scores),
                # then v (feeds PV), then the 4 q heads
                srcs = {}
                for slot in (4, 5, 0, 1, 2, 3):
                    ps = psA.tile([P, 512], f32, tag="proj")
                    for j in range(NK):
                        if slot < 4:
                            lhs = wq_sb[:, j, slot * HD:(slot + 1) * HD]
                        elif slot == 4:
                            lhs = wk_sb[:, j, :]
                        else:
                            lhs = wv_sb[:, j, :]
                        nc.tensor.matmul(ps, lhs, xt_c[:, j, :],
                                         start=(j == 0), stop=(j == NK - 1))
                    if slot == 5:
                        vT_c = vTp.tile([P, 512], bf16, tag="vT")
                        nc.scalar.copy(vT_c, ps)
                        for u in range(4):
                            tp = psA2.tile([P, HD], bf16, tag="vtr")
                            nc.tensor.transpose(tp, vT_c[:, u * HD:(u + 1) * HD], ident)
                            nc.scalar.copy(v_nat[:, 4 * c + u, :], tp)
                    else:
                        t32 = q32p.tile([P, 512], bf16, tag="q32",
                                        name=f"q32_{c}_{slot}")
                        nc.scalar.copy(t32, ps)
                        srcs[slot] = t32
                # rmsnorm + rope for this chunk (k first)
                for t in (4, 0, 1, 2, 3):
                    src = srcs[t]
                    dst = qhat[t] if t < 4 else khat
                    cosT = cs_sb["cosq" if t < 4 else "cosk"]
                    sinT = cs_sb["sinq" if t < 4 else "sink"]
                    sqb = scr.tile([P, 512], bf16, tag="sqb")
                    nc.scalar.activation(sqb, src, AF.Square)
                    ssq = psA.tile([P, 512], f32, tag="proj")
                    nc.tensor.matmul(ssq, osb, sqb, start=True, stop=True)
                    msinv = scr.tile([P, 512], f32, tag="msinv")
                    nc.vector.reciprocal_approx_fast(out=msinv, in_=ssq)
                    rsb = scr.tile([P, 512], bf16, tag="rsb")
                    nc.scalar.activation(rsb, msinv, AF.Sqrt)
                    rot = psA2.tile([P, 512], f32, tag="rot")
                    nc.tensor.matmul(rot, rsw_sb, src, start=True, stop=True)
                    rot_sb = scr.tile([P, 512], bf16, tag="rot_sb")
                    nc.scalar.copy(rot_sb, rot)
                    t1 = scr.tile([P, 512], bf16, tag="t1")
                    nc.vector.tensor_mul(t1, src, cosT[:, sl])
                    t2 = scr.tile([P, 512], bf16, tag="t2")
                    nc.vector.tensor_mul(t2, rot_sb, sinT[:, sl])
                    t3 = scr.tile([P, 512], bf16, tag="t3")
                    nc.vector.tensor_add(t3, t1, t2)
                    nc.vector.tensor_mul(dst[:, sl], t3, rsb)

        # ---- phase B: attention + output projection, per chunk ----
        with tc.tile_pool(name="ptp", bufs=16) as ptp, \
             tc.tile_pool(name="accp", bufs=2) as accp, \
             tc.tile_pool(name="recp", bufs=2) as recp, \
             tc.tile_pool(name="rowp", bufs=2) as rowp, \
             tc.tile_pool(name="psc", bufs=2, space="PSUM") as psc, \
             tc.tile_pool(name="pss", bufs=4, space="PSUM") as pss:
            for c in range(NCH):
                sl = slice(c * 512, (c + 1) * 512)
                for h in range(GQ):
                    # scores -> exp -> (mask) -> probability tiles
                    # off-diagonal key-tile pairs: full 512-query width
                    pv_list = []
                    for pr in range(2 * c):
                        sc = psc.tile([P, 1024], f32, tag="sc",
                                      name=f"sc_{c}_{h}_{pr}")
                        for u in range(2):
                            j = 2 * pr + u
                            nc.tensor.matmul(sc[:, u * 512:(u + 1) * 512],
                                             khat[:, j * P:(j + 1) * P],
                                             qhat[h][:, sl],
                                             start=True, stop=True)
                        pt = ptp.tile([P, 1024], bf16, tag="pt",
                                      name=f"pt_{c}_{h}_{pr}")
                        nc.scalar.activation(pt, sc, AF.Exp, scale=inv_sqrt_hd)
                        pv_list.append((2 * pr, pt, 0, 0, 512))
                        pv_list.append((2 * pr + 1, pt, 512, 0, 512))
                    # diagonal 512-block: restricted query ranges
                    # tile u covers queries [128u, 512) of the chunk
                    scA = psc.tile([P, 1024], f32, tag="sc", name=f"scA_{c}_{h}")
                    nc.tensor.matmul(scA[:, 0:512],
                                     khat[:, (4 * c) * P:(4 * c + 1) * P],
                                     qhat[h][:, c * 512:(c + 1) * 512],
                                     start=True, stop=True)
                    nc.tensor.matmul(scA[:, 512:896],
                                     khat[:, (4 * c + 1) * P:(4 * c + 2) * P],
                                     qhat[h][:, c * 512 + 128:(c + 1) * 512],
                                     start=True, stop=True)
                    ptA = ptp.tile([P, 1024], bf16, tag="pt", name=f"ptA_{c}_{h}")
                    nc.scalar.activation(ptA[:, 0:896], scA[:, 0:896],
                                         AF.Exp, scale=inv_sqrt_hd)
                    nc.vector.tensor_mul(ptA[:, 0:896], ptA[:, 0:896],
                                         mtri_sb[:, 0:896])
                    scB = psc.tile([P, 1024], f32, tag="sc", name=f"scB_{c}_{h}")
                    nc.tensor.matmul(scB[:, 0:256],
                                     khat[:, (4 * c + 2) * P:(4 * c + 3) * P],
                                     qhat[h][:, c * 512 + 256:(c + 1) * 512],
                                     start=True, stop=True)
                    nc.tensor.matmul(scB[:, 256:384],
                                     khat[:, (4 * c + 3) * P:(4 * c + 4) * P],
                                     qhat[h][:, c * 512 + 384:(c + 1) * 512],
                                     start=True, stop=True)
                    ptB = ptp.tile([P, 1024], bf16, tag="pt", name=f"ptB_{c}_{h}")
                    nc.scalar.activation(ptB[:, 0:384], scB[:, 0:384],
                                         AF.Exp, scale=inv_sqrt_hd)
                    nc.vector.tensor_mul(ptB[:, 0:384], ptB[:, 0:384],
                                         mtri_sb[:, 896:1280])
                    pv_list.append((4 * c + 0, ptA, 0, 0, 512))
                    pv_list.append((4 * c + 1, ptA, 512, 128, 384))
                    pv_list.append((4 * c + 2, ptB, 0, 256, 256))
                    pv_list.append((4 * c + 3, ptB, 256, 384, 128))

                    # denominator: accumulate probability tiles on DVE,
                    # then one ones-matmul partition reduction
                    acc = accp.tile([P, 512], bf16, tag="acc", name=f"acc_{c}_{h}")
                    first = True
                    for (_, pt, co, qo, w) in pv_list:
                        if first:
                            nc.vector.tensor_copy(acc, pt[:, co:co + w])
                            first = False
                        else:
                            nc.vector.tensor_add(acc[:, qo:qo + w],
                                                 acc[:, qo:qo + w],
                                                 pt[:, co:co + w])
                    den = pss.tile([P, 512], f32, tag="s", name=f"den_{c}_{h}")
                    nc.tensor.matmul(den, ones_sb, acc, start=True, stop=True)
                    rec = recp.tile([P, 512], f32, tag="rec")
                    nc.vector.reciprocal_approx_fast(out=rec, in_=den)

                    # P@V for this head, j-ordered accumulation
                    ots = pss.tile([P, 512], f32, tag="s", name=f"ot_{c}_{h}")
                    n_pv = len(pv_list)
                    for idx, (j, pt, co, qo, w) in enumerate(pv_list):
                        nc.tensor.matmul(ots[:, qo:qo + w], v_nat[:, j, :],
                                         pt[:, co:co + w],
                                         start=(idx == 0), stop=(idx == n_pv - 1))
                    nc.vector.tensor_mul(onorm[h][:, sl], ots, rec)

                # output projection for this chunk's 4 token tiles
                for i in range(4 * c, 4 * c + 4):
                    isl = slice(i * P, (i + 1) * P)
                    row = rowp.tile([P, DIM], bf16, tag="row", name=f"row_{i}")
                    for n in range(NCH):
                        po_ps = pss.tile([P, 512], f32, tag="s",
                                         name=f"po_{i}_{n}")
                        for h in range(GQ):
                            nc.tensor.matmul(po_ps, onorm[h][:, isl],
                                             wo_sb[:, h, n * 512:(n + 1) * 512],
                                             start=(h == 0), stop=(h == GQ - 1))
                        nc.vector.tensor_copy(row[:, n * 512:(n + 1) * 512], po_ps)
                    eng = nc.sync if i % 2 == 0 else nc.scalar
                    eng.dma_start(out=po[isl, :], in_=row)
    nc.compile()
    return nc


def _causal_ok(mask):
    m = np.asarray(mask).reshape(S, S)
    tri = np.tril(np.ones((S, S), dtype=bool))
    return bool(np.all(m[tri] == 0.0) and np.all(m[~tri] <= -1e8))


def _reference_fallback(x, Wq, Wk, Wv, Wo, qg, kg, cos, sin, mask):
    x64 = np.asarray(x, dtype=np.float32)
    q = (x64 @ Wq).reshape(B, S, H, HD).transpose(0, 2, 1, 3)
    k = (x64 @ Wk).reshape(B, S, KV, HD).transpose(0, 2, 1, 3)
    v = (x64 @ Wv).reshape(B, S, KV, HD).transpose(0, 2, 1, 3)

    def rms(t, g):
        r = np.sqrt(np.mean(t * t, axis=-1, keepdims=True) + EPS)
        return g * (t / r)

    q, k = rms(q, qg), rms(k, kg)

    def rot(t):
        return np.concatenate([-t[..., HD // 2:], t[..., :HD // 2]], axis=-1)

    c = cos[None, None, :, :]
    s = sin[None, None, :, :]
    q = q * c + rot(q) * s
    k = k * c + rot(k) * s
    k = np.repeat(k, GQ, axis=1)
    v = np.repeat(v, GQ, axis=1)
    sc = np.einsum('bhqd,bhkd->bhqk', q, k) / np.sqrt(HD) + np.asarray(mask).reshape(1, 1, S, S)
    sc = sc - sc.max(axis=-1, keepdims=True)
    e = np.exp(sc)
    a = e / e.sum(axis=-1, keepdims=True)
    o = np.einsum('bhqk,bhkd->bhqd', a, v)
    o = o.transpose(0, 2, 1, 3).reshape(B, S, H * HD)
    return (o @ Wo).astype(np.float32)


def kernel(x, Wq, Wk, Wv, Wo, qg, kg, cos, sin, mask, **_unused):
    x = np.asarray(x, dtype=np.float32)
    Wq, Wk, Wv, Wo = (np.asarray(a, dtype=np.float32) for a in (Wq, Wk, Wv, Wo))
    qg, kg = np.asarray(qg, np.float32), np.asarray(kg, np.float32)
    cos, sin = np.asarray(cos, np.float32), np.asarray(sin, np.float32)
    if not _causal_ok(mask):
        return _reference_fallback(x, Wq, Wk, Wv, Wo, qg, kg, cos, sin, mask)

    from concourse.bass_utils import run_bass_kernel_spmd

    if "nc" not in _CACHED:
        _CACHED["nc"] = _build_program()
    nc = _CACHED["nc"]

    cosT = np.ascontiguousarray(cos.T)  # [HD, S]
    sinT = np.ascontiguousarray(sin.T)

    # rope via halves: out[:64] = x[:64]*cos[:64] + x[64:]*sin_tbl[:64]
    #                  out[64:] = x[64:]*cos[64:] + x[:64]*sin_tbl[64:]
    # reference: rot(x)[:64] = -x[64:], rot(x)[64:] = x[:64]; gains fold in.
    def tables(g):
        ct = cosT * g[:, None]
        st = np.empty_like(sinT)
        st[:64] = -sinT[:64] * g[64:, None]
        st[64:] = sinT[64:] * g[:64, None]
        return ct.astype(BF), st.astype(BF)

    cq, sq = tables(qg)
    ck, sk = tables(kg)

    rsw = np.zeros((P, P), dtype=np.float32)
    for i in range(P):
        rsw[i, (i + 64) % P] = 1.0
    rsw = rsw.astype(BF)

    # restricted-diagonal masks: within each 128-column sub-range that
    # starts a diagonal tile, query-col >= key-row; elsewhere 1.
    rows = np.arange(P)[:, None]
    tri = (np.arange(P)[None, :] >= rows)          # [128,128] step
    onesP = np.ones((P, P), dtype=bool)
    mA = np.concatenate([tri, onesP, onesP, onesP, tri, onesP, onesP], axis=1)  # 896
    mB = np.concatenate([tri, onesP, tri], axis=1)                              # 384
    mtri = np.concatenate([mA, mB], axis=1).astype(BF)                          # [128,1280]

    def part_layout(w, cols):
        # [DIM, cols] -> [P, NK, cols] with feature d = j*128 + p
        return np.ascontiguousarray(w.reshape(NK, P, cols).transpose(1, 0, 2)).astype(BF)

    xt4 = []
    for b in range(B):
        xT = x[b].T  # [DIM, S]
        xt4.append(np.ascontiguousarray(
            xT.reshape(NK, P, NCH, 512).transpose(1, 2, 0, 3)).astype(BF))

    in_maps = []
    for core in range(8):
        b, g = divmod(core, KV)
        wo_g = Wo[g * GQ * HD:(g + 1) * GQ * HD, :]
        in_maps.append({
            "xt4": xt4[b],
            "wq": part_layout(Wq[:, g * GQ * HD:(g + 1) * GQ * HD], GQ * HD),
            "wk": part_layout(Wk[:, g * HD:(g + 1) * HD], HD),
            "wv": part_layout(Wv[:, g * HD:(g + 1) * HD], HD),
            "wo": np.ascontiguousarray(
                wo_g.reshape(GQ, P, DIM).transpose(1, 0, 2)).astype(BF),
            "cosq": cq, "sinq": sq, "cosk": ck, "sink": sk,
            "mtri": mtri, "rsw": rsw,
        })

    res = run_bass_kernel_spmd(nc, in_maps, list(range(8)))
    out = np.zeros((B, S, DIM), dtype=np.float32)
    for core in range(8):
        out[core // KV] += res.results[core]["po"].astype(np.float32)
    return out


# revision 16
# speedup vs baseline: 1.3197x; 1.1412x over previous
"""GroupedQueryAttention Trainium2 kernel (8 NeuronCores).

Sharding: (batch b in 0..1) x (kv-head group g in 0..3) -> core 4*b+g.
Each core computes, for its batch, the 4 query heads (4g..4g+3) that share
kv head g, plus the partial output projection through the matching 512-row
slice of Wo.  The host sums the 4 bf16 partials per batch in f32.

On-device dataflow is fully "transposed": activations live as [feature,
token] so every matmul contraction sits on the partition axis, and the
softmax probabilities come out directly in the layout the P@V matmul
needs.  Performance structure vs the phase-serial baseline:
  - host pre-arranges every DRAM operand so each DMA is contiguous per
    partition; x arrives in per-chunk pieces on the sync queue while the
    weight tensors stream on the scalar queue
  - dummy warm-up matmuls run during the initial DMA wait so the PE HAM
    clock gate is at 8/8 when real work starts
  - causal diagonal 512-blocks are computed on restricted query ranges
    (512/384/256/128 wide) instead of full width + mask
  - softmax denominators are accumulated on the Vector engine (bf16 adds
    over the probability tiles) with a single ones-matmul partition
    reduction per (chunk, head); 1/den uses reciprocal_approx_fast
  - rmsnorm uses reciprocal_approx_fast + a Sqrt activation, keeping the
    Scalar engine on two activation-table sets total (sqrt phase, exp
    phase) instead of thrashing Square/Ln/Exp loads
  - the output projection is interleaved per chunk with attention, with
    PSUM pools sized so attention + projection coexist in the 8 banks;
    bf16 rows stream out as soon as each 128-token tile is projected
"""

import numpy as np
import ml_dtypes

DIM, H, KV, S, B = 2048, 16, 4, 2048, 2
HD = DIM // H          # 128
GQ = H // KV           # 4 query heads per kv head
P = 128                # partitions
NK = DIM // P          # 16 contraction tiles
NCH = S // 512         # 4 sequence chunks of 512
EPS = 1e-6
BF = ml_dtypes.bfloat16

_CACHED = {}


def _build_program():
    import concourse.bass as bass
    import concourse.tile as tile
    from concourse import bacc
    from concourse import mybir
    from concourse.masks import make_identity

    f32 = mybir.dt.float32
    bf16 = mybir.dt.bfloat16
    AF = mybir.ActivationFunctionType

    nc = bacc.Bacc()
    xt4 = nc.declare_dram_parameter("xt4", [P, NCH, NK, 512], bf16, isOutput=False)
    wq = nc.declare_dram_parameter("wq", [P, NK, GQ * HD], bf16, isOutput=False)
    wk = nc.declare_dram_parameter("wk", [P, NK, HD], bf16, isOutput=False)
    wv = nc.declare_dram_parameter("wv", [P, NK, HD], bf16, isOutput=False)
    wo = nc.declare_dram_parameter("wo", [P, GQ, DIM], bf16, isOutput=False)
    cosq = nc.declare_dram_parameter("cosq", [HD, S], bf16, isOutput=False)
    sinq = nc.declare_dram_parameter("sinq", [HD, S], bf16, isOutput=False)
    cosk = nc.declare_dram_parameter("cosk", [HD, S], bf16, isOutput=False)
    sink = nc.declare_dram_parameter("sink", [HD, S], bf16, isOutput=False)
    mtri = nc.declare_dram_parameter("mtri", [P, 1280], bf16, isOutput=False)
    rsw = nc.declare_dram_parameter("rsw", [P, P], bf16, isOutput=False)
    po = nc.declare_dram_parameter("po", [S, DIM], bf16, isOutput=True)

    inv_sqrt_hd = 1.0 / float(np.sqrt(HD))

    with tile.TileContext(nc) as tc:
      with tc.tile_pool(name="const", bufs=1) as const, \
           tc.tile_pool(name="w5", bufs=1) as w5, \
           tc.tile_pool(name="hatp", bufs=1) as hatp:
        ones_sb = const.tile([P, P], bf16)
        nc.vector.memset(ones_sb, 1.0)
        osb = const.tile([P, P], bf16)           # 1/HD for the rmsnorm mean
        nc.vector.memset(osb, 1.0 / HD)
        ident = const.tile([P, P], bf16)
        make_identity(nc, ident)
        wmov = const.tile([P, 512], bf16)
        nc.vector.memset(wmov, 0.0)
        mtri_sb = const.tile([P, 1280], bf16)
        nc.gpsimd.dma_start(out=mtri_sb, in_=mtri[:, :])
        sq_dummy = const.tile([P, 1], f32)
        nc.vector.memset(sq_dummy, 1.0)
        sq_dummy_o = const.tile([P, 1], bf16)
        # preload the sqrt activation-table set during the DMA wait
        nc.scalar.activation(sq_dummy_o, sq_dummy, AF.Sqrt)

        wo_sb = w5.tile([P, GQ, DIM], bf16)
        nc.gpsimd.dma_start(out=wo_sb, in_=wo.ap().rearrange("p h n -> p (h n)"))

        v_nat = hatp.tile([P, NK, HD], bf16, tag="vnat")
        khat = hatp.tile([P, S], bf16, tag="khat")
        qhat = [hatp.tile([P, S], bf16, tag=f"qhat{h}", name=f"qhat{h}")
                for h in range(GQ)]
        onorm = [hatp.tile([P, S], bf16, tag=f"onorm{h}", name=f"onorm{h}")
                 for h in range(GQ)]

        # ---- warm-up: keep the PE busy while the first DMAs land ----
        with tc.tile_pool(name="wps", bufs=1, space="PSUM") as wps:
            wt = wps.tile([P, 512], f32, tag="warm")
            for _ in range(14):
                nc.tensor.matmul(wt, ones_sb, wmov, start=True, stop=True)

        # ---- phase A: projections + rmsnorm + rope, chunk-pipelined ----
        with tc.tile_pool(name="xtp", bufs=1) as xtp, \
             tc.tile_pool(name="xchk", bufs=2) as xchk, \
             tc.tile_pool(name="q32p", bufs=12) as q32p, \
             tc.tile_pool(name="vTp", bufs=2) as vTp, \
             tc.tile_pool(name="scr", bufs=2) as scr, \
             tc.tile_pool(name="psA", bufs=3, space="PSUM") as psA, \
             tc.tile_pool(name="psQ", bufs=2, space="PSUM") as psQ, \
             tc.tile_pool(name="psA2", bufs=2, space="PSUM") as psA2:
            # sync's DMA queue is several times slower than the scalar and
            # gpsimd queues, so everything latency-critical goes on those two
            wk_sb = xtp.tile([P, NK, HD], bf16, tag="wk")
            nc.scalar.dma_start(out=wk_sb, in_=wk.ap().rearrange("p j n -> p (j n)"))
            wq_sb = xtp.tile([P, NK, GQ * HD], bf16, tag="wq")
            wv_sb = xtp.tile([P, NK, HD], bf16, tag="wv")
            rsw_sb = xtp.tile([P, P], bf16, tag="rsw")
            nc.gpsimd.dma_start(out=rsw_sb, in_=rsw[:, :])
            cs_sb = {}
            for nm, t in (("cosq", cosq), ("sinq", sinq), ("cosk", cosk), ("sink", sink)):
                cs_sb[nm] = xtp.tile([P, S], bf16, tag=f"cs_{nm}", name=f"cs_{nm}")

            def dma_chunk(c):
                xt_c = xchk.tile([P, NK, 512], bf16, tag="xt", name=f"xt{c}")
                xsrc = xt4.ap()[:, c]
                nc.scalar.dma_start(out=xt_c[:, 0:8, :], in_=xsrc[:, 0:8])
                nc.gpsimd.dma_start(out=xt_c[:, 8:16, :], in_=xsrc[:, 8:16])
                return xt_c

            xts = {0: dma_chunk(0)}
            nc.scalar.dma_start(out=wq_sb, in_=wq.ap().rearrange("p j n -> p (j n)"))
            nc.scalar.dma_start(out=wv_sb, in_=wv.ap().rearrange("p j n -> p (j n)"))
            for nm, t in (("cosq", cosq), ("sinq", sinq), ("cosk", cosk), ("sink", sink)):
                nc.gpsimd.dma_start(out=cs_sb[nm], in_=t[:, :])

            def p1(c):
                xt_c = xts.pop(c)
                srcs = {}
                for slot in (4, 5, 0, 1, 2, 3):
                    ps = psA.tile([P, 512], f32, tag="proj")
                    for j in range(NK):
                        if slot < 4:
                            lhs = wq_sb[:, j, slot * HD:(slot + 1) * HD]
                        elif slot == 4:
                            lhs = wk_sb[:, j, :]
                        else:
                            lhs = wv_sb[:, j, :]
                        nc.tensor.matmul(ps, lhs, xt_c[:, j, :],
                                         start=(j == 0), stop=(j == NK - 1))
                    if slot == 5:
                        vT_c = vTp.tile([P, 512], bf16, tag="vT")
                        nc.scalar.copy(vT_c, ps)
                        tp = psA2.tile([P, 512], bf16, tag="vtr", bufs=1)
                        for u in range(4):
                            nc.tensor.transpose(tp[:, u * HD:(u + 1) * HD],
                                                vT_c[:, u * HD:(u + 1) * HD], ident)
                        nc.scalar.copy(v_nat[:, 4 * c:4 * c + 4, :], tp)
                    else:
                        t32 = q32p.tile([P, 512], bf16, tag="q32",
                                        name=f"q32_{c}_{slot}")
                        nc.scalar.copy(t32, ps)
                        srcs[slot] = t32
                return srcs

            def p2(c, srcs):
                sl = slice(c * 512, (c + 1) * 512)
                for t in (4, 0, 1, 2, 3):
                    src = srcs[t]
                    dst = qhat[t] if t < 4 else khat
                    cosT = cs_sb["cosq" if t < 4 else "cosk"]
                    sinT = cs_sb["sinq" if t < 4 else "sink"]
                    sqb = scr.tile([P, 512], bf16, tag="sqb")
                    nc.scalar.activation(sqb, src, AF.Square)
                    ssq = psQ.tile([P, 512], f32, tag="ssq")
                    nc.tensor.matmul(ssq, osb, sqb, start=True, stop=True)
                    msinv = scr.tile([P, 512], f32, tag="msinv")
                    nc.vector.reciprocal_approx_fast(out=msinv, in_=ssq)
                    rsb = scr.tile([P, 512], bf16, tag="rsb")
                    nc.scalar.activation(rsb, msinv, AF.Sqrt)
                    rot = psA2.tile([P, 512], f32, tag="rot")
                    nc.tensor.matmul(rot, rsw_sb, src, start=True, stop=True)
                    rot_sb = scr.tile([P, 512], bf16, tag="rot_sb")
                    nc.scalar.copy(rot_sb, rot)
                    t1 = scr.tile([P, 512], bf16, tag="t1")
                    nc.vector.tensor_mul(t1, src, cosT[:, sl])
                    t2 = scr.tile([P, 512], bf16, tag="t2")
                    nc.vector.tensor_mul(t2, rot_sb, sinT[:, sl])
                    t3 = scr.tile([P, 512], bf16, tag="t3")
                    nc.vector.tensor_add(t3, t1, t2)
                    nc.vector.tensor_mul(dst[:, sl], t3, rsb)

            # software-pipelined: P2 for chunk c-1 is emitted after P1 for
            # chunk c, so its small matmuls never head-of-line block P1
            prev = None
            for c in range(NCH):
                if c + 1 < NCH:
                    xts[c + 1] = dma_chunk(c + 1)
                cur = (c, p1(c))
                if prev is not None:
                    p2(*prev)
                prev = cur
            p2(*prev)

        # ---- phase B: attention + output projection, per chunk ----
        with tc.tile_pool(name="ptp", bufs=34) as ptp, \
             tc.tile_pool(name="accp", bufs=4) as accp, \
             tc.tile_pool(name="recp", bufs=2) as recp, \
             tc.tile_pool(name="rowp", bufs=2) as rowp, \
             tc.tile_pool(name="psc", bufs=2, space="PSUM") as psc, \
             tc.tile_pool(name="pss", bufs=1, space="PSUM") as pss:
            for c in range(NCH):
                sl = slice(c * 512, (c + 1) * 512)
                # pass 1: scores -> exp -> mask -> DVE denominator adds for
                # all four heads; pass 2 (den matmul, P@V, normalize) follows
                # so the tensor queue always has ready work to pull
                pvs = {}
                accs = {}
                for h in range(GQ):
                    # off-diagonal key-tile pairs: full 512-query width
                    pv_list = []
                    for pr in range(2 * c):
                        sc = psc.tile([P, 1024], f32, tag="sc",
                                      name=f"sc_{c}_{h}_{pr}")
                        for u in range(2):
                            j = 2 * pr + u
                            nc.tensor.matmul(sc[:, u * 512:(u + 1) * 512],
                                             khat[:, j * P:(j + 1) * P],
                                             qhat[h][:, sl],
                                             start=True, stop=True)
                        pt = ptp.tile([P, 1024], bf16, tag="pt",
                                      name=f"pt_{c}_{h}_{pr}")
                        nc.scalar.activation(pt, sc, AF.Exp, scale=inv_sqrt_hd)
                        pv_list.append((2 * pr, pt, 0, 0, 512))
                        pv_list.append((2 * pr + 1, pt, 512, 0, 512))
                    # diagonal 512-block: restricted query ranges
                    # tile u covers queries [128u, 512) of the chunk
                    scA = psc.tile([P, 1024], f32, tag="sc", name=f"scA_{c}_{h}")
                    nc.tensor.matmul(scA[:, 0:512],
                                     khat[:, (4 * c) * P:(4 * c + 1) * P],
                                     qhat[h][:, c * 512:(c + 1) * 512],
                                     start=True, stop=True)
                    nc.tensor.matmul(scA[:, 512:896],
                                     khat[:, (4 * c + 1) * P:(4 * c + 2) * P],
                                     qhat[h][:, c * 512 + 128:(c + 1) * 512],
                                     start=True, stop=True)
                    ptA = ptp.tile([P, 1024], bf16, tag="pt", name=f"ptA_{c}_{h}")
                    nc.scalar.activation(ptA[:, 0:896], scA[:, 0:896],
                                         AF.Exp, scale=inv_sqrt_hd)
                    nc.vector.tensor_mul(ptA[:, 0:896], ptA[:, 0:896],
                                         mtri_sb[:, 0:896])
                    scB = psc.tile([P, 1024], f32, tag="sc", name=f"scB_{c}_{h}")
                    nc.tensor.matmul(scB[:, 0:256],
                                     khat[:, (4 * c + 2) * P:(4 * c + 3) * P],
                                     qhat[h][:, c * 512 + 256:(c + 1) * 512],
                                     start=True, stop=True)
                    nc.tensor.matmul(scB[:, 256:384],
                                     khat[:, (4 * c + 3) * P:(4 * c + 4) * P],
                                     qhat[h][:, c * 512 + 384:(c + 1) * 512],
                                     start=True, stop=True)
                    ptB = ptp.tile([P, 1024], bf16, tag="pt", name=f"ptB_{c}_{h}")
                    nc.scalar.activation(ptB[:, 0:384], scB[:, 0:384],
                                         AF.Exp, scale=inv_sqrt_hd)
                    nc.vector.tensor_mul(ptB[:, 0:384], ptB[:, 0:384],
                                         mtri_sb[:, 896:1280])
                    pv_list.append((4 * c + 0, ptA, 0, 0, 512))
                    pv_list.append((4 * c + 1, ptA, 512, 128, 384))
                    pv_list.append((4 * c + 2, ptB, 0, 256, 256))
                    pv_list.append((4 * c + 3, ptB, 256, 384, 128))
                    pvs[h] = pv_list

                    # denominator: accumulate probability tiles on DVE
                    acc = accp.tile([P, 512], bf16, tag="acc", name=f"acc_{c}_{h}")
                    first = True
                    for (_, pt, co, qo, w) in pv_list:
                        if first:
                            nc.vector.tensor_copy(acc, pt[:, co:co + w])
                            first = False
                        else:
                            nc.vector.tensor_add(acc[:, qo:qo + w],
                                                 acc[:, qo:qo + w],
                                                 pt[:, co:co + w])
                    accs[h] = acc

                # pass 2: partition-reduce den, P@V, normalize
                for h in range(GQ):
                    pv_list = pvs[h]
                    den = pss.tile([P, 512], f32, tag="den", name=f"den_{c}_{h}")
                    nc.tensor.matmul(den, ones_sb, accs[h], start=True, stop=True)
                    rec = recp.tile([P, 512], f32, tag="rec")
                    nc.vector.reciprocal_approx_fast(out=rec, in_=den)

                    ots = pss.tile([P, 512], f32, tag="ots", bufs=2,
                                   name=f"ot_{c}_{h}")
                    n_pv = len(pv_list)
                    for idx, (j, pt, co, qo, w) in enumerate(pv_list):
                        nc.tensor.matmul(ots[:, qo:qo + w], v_nat[:, j, :],
                                         pt[:, co:co + w],
                                         start=(idx == 0), stop=(idx == n_pv - 1))
                    nc.vector.tensor_mul(onorm[h][:, sl], ots, rec)

                # output projection for this chunk's 4 token tiles
                for i in range(4 * c, 4 * c + 4):
                    isl = slice(i * P, (i + 1) * P)
                    row = rowp.tile([P, DIM], bf16, tag="row", name=f"row_{i}")
                    for n in range(NCH):
                        po_ps = pss.tile([P, 512], f32, tag="po",
                                         name=f"po_{i}_{n}")
                        for h in range(GQ):
                            nc.tensor.matmul(po_ps, onorm[h][:, isl],
                                             wo_sb[:, h, n * 512:(n + 1) * 512],
                                             start=(h == 0), stop=(h == GQ - 1))
                        nc.vector.tensor_copy(row[:, n * 512:(n + 1) * 512], po_ps)
                    eng = nc.scalar if c == NCH - 1 else nc.sync
                    eng.dma_start(out=po[isl, :], in_=row)
    nc.compile()
    return nc


def _causal_ok(mask):
    m = np.asarray(mask).reshape(S, S)
    tri = np.tril(np.ones((S, S), dtype=bool))
    return bool(np.all(m[tri] == 0.0) and np.all(m[~tri] <= -1e8))


def _reference_fallback(x, Wq, Wk, Wv, Wo, qg, kg, cos, sin, mask):
    x64 = np.asarray(x, dtype=np.float32)
    q = (x64 @ Wq).reshape(B, S, H, HD).transpose(0, 2, 1, 3)
    k = (x64 @ Wk).reshape(B, S, KV, HD).transpose(0, 2, 1, 3)
    v = (x64 @ Wv).reshape(B, S, KV, HD).transpose(0, 2, 1, 3)

    def rms(t, g):
        r = np.sqrt(np.mean(t * t, axis=-1, keepdims=True) + EPS)
        return g * (t / r)

    q, k = rms(q, qg), rms(k, kg)

    def rot(t):
        return np.concatenate([-t[..., HD // 2:], t[..., :HD // 2]], axis=-1)

    c = cos[None, None, :, :]
    s = sin[None, None, :, :]
    q = q * c + rot(q) * s
    k = k * c + rot(k) * s
    k = np.repeat(k, GQ, axis=1)
    v = np.repeat(v, GQ, axis=1)
    sc = np.einsum('bhqd,bhkd->bhqk', q, k) / np.sqrt(HD) + np.asarray(mask).reshape(1, 1, S, S)
    sc = sc - sc.max(axis=-1, keepdims=True)
    e = np.exp(sc)
    a = e / e.sum(axis=-1, keepdims=True)
    o = np.einsum('bhqk,bhkd->bhqd', a, v)
    o = o.transpose(0, 2, 1, 3).reshape(B, S, H * HD)
    return (o @ Wo).astype(np.float32)


def kernel(x, Wq, Wk, Wv, Wo, qg, kg, cos, sin, mask, **_unused):
    x = np.asarray(x, dtype=np.float32)
    Wq, Wk, Wv, Wo = (np.asarray(a, dtype=np.float32) for a in (Wq, Wk, Wv, Wo))
    qg, kg = np.asarray(qg, np.float32), np.asarray(kg, np.float32)
    cos, sin = np.asarray(cos, np.float32), np.asarray(sin, np.float32)
    if not _causal_ok(mask):
        return _reference_fallback(x, Wq, Wk, Wv, Wo, qg, kg, cos, sin, mask)

    from concourse.bass_utils import run_bass_kernel_spmd

    if "nc" not in _CACHED:
        _CACHED["nc"] = _build_program()
    nc = _CACHED["nc"]

    cosT = np.ascontiguousarray(cos.T)  # [HD, S]
    sinT = np.ascontiguousarray(sin.T)

    # rope via halves: out[:64] = x[:64]*cos[:64] + x[64:]*sin_tbl[:64]
    #                  out[64:] = x[64:]*cos[64:] + x[:64]*sin_tbl[64:]
    # reference: rot(x)[:64] = -x[64:], rot(x)[64:] = x[:64]; gains fold in.
    def tables(g):
        ct = cosT * g[:, None]
        st = np.empty_like(sinT)
        st[:64] = -sinT[:64] * g[64:, None]
        st[64:] = sinT[64:] * g[:64, None]
        return ct.astype(BF), st.astype(BF)

    cq, sq = tables(qg)
    ck, sk = tables(kg)

    rsw = np.zeros((P, P), dtype=np.float32)
    for i in range(P):
        rsw[i, (i + 64) % P] = 1.0
    rsw = rsw.astype(BF)

    # restricted-diagonal masks: within each 128-column sub-range that
    # starts a diagonal tile, query-col >= key-row; elsewhere 1.
    rows = np.arange(P)[:, None]
    tri = (np.arange(P)[None, :] >= rows)          # [128,128] step
    onesP = np.ones((P, P), dtype=bool)
    mA = np.concatenate([tri, onesP, onesP, onesP, tri, onesP, onesP], axis=1)  # 896
    mB = np.concatenate([tri, onesP, tri], axis=1)                              # 384
    mtri = np.concatenate([mA, mB], axis=1).astype(BF)                          # [128,1280]

    def part_layout(w, cols):
        # [DIM, cols] -> [P, NK, cols] with feature d = j*128 + p
        return np.ascontiguousarray(w.reshape(NK, P, cols).transpose(1, 0, 2)).astype(BF)

    xt4 = []
    for b in range(B):
        xT = x[b].T  # [DIM, S]
        xt4.append(np.ascontiguousarray(
            xT.reshape(NK, P, NCH, 512).transpose(1, 2, 0, 3)).astype(BF))

    in_maps = []
    for core in range(8):
        b, g = divmod(core, KV)
        wo_g = Wo[g * GQ * HD:(g + 1) * GQ * HD, :]
        in_maps.append({
            "xt4": xt4[b],
            "wq": part_layout(Wq[:, g * GQ * HD:(g + 1) * GQ * HD], GQ * HD),
            "wk": part_layout(Wk[:, g * HD:(g + 1) * HD], HD),
            "wv": part_layout(Wv[:, g * HD:(g + 1) * HD], HD),
            "wo": np.ascontiguousarray(
                wo_g.reshape(GQ, P, DIM).transpose(1, 0, 2)).astype(BF),
            "cosq": cq, "sinq": sq, "cosk": ck, "sink": sk,
            "mtri": mtri, "rsw": rsw,
        })

    res = run_bass_kernel_spmd(nc, in_maps, list(range(8)))
    out = np.zeros((B, S, DIM), dtype=np.float32)
    for core in range(8):
        out[core // KV] += res.results[core]["po"].astype(np.float32)
    return out
